# revision 15
# baseline (speedup 1.0000x reference)
"""COOTensorProduct kernel for 8 Trainium2 NeuronCores.

Math: out[b, h] = sum_{i,j} cb[h, i*64+j] * in1[b, i] * in2[b, j]
with in1/in2 [4096, 64], cb [4096, 4096] (a Clebsch-Gordan / Wigner-3j
coupling matrix for irreps '4x0e+4x1o+4x2e+4x3o' x same -> all l3).

cb is 0.1% dense but perfectly block-structured: for each (l1, l2) pair
of irrep types the coupling is a square (2l1+1)(2l2+1) x (2l1+1)(2l2+1)
matrix (stacked l3 blocks), identical across the 4x4 multiplicity copies
(u, v). The 16 pair matrices have sizes {1,3,3,5,5,7,7,9,15,15,21,21,
25,35,35,49} which pack block-diagonally into exactly two 128x128
stationary matrices (49+35+35+9 = 128 and the rest = 128).

Per core (512 batch rows):
  rhs[S][u,v]  = in1T_gathered[S,u] * in2T_gathered[S,v]   (elementwise,
                 [128 partitions = (pair,m1,m2) rows, 512 free = batch])
  psum[S][u,v] = W_S.T @ rhs        (one 128x128x512 matmul)
so the whole problem is 32 elementwise mults + 32 matmuls per core.

Host does the (static, index-only) gathers/permutes; device does all
FLOPs. Output comes back as [4096 permuted rows, 512 batch] per core and
is un-permuted/transposed on host.
"""

import json
import numpy as np

# ---------------------------------------------------------------- problem
B = 4096
DIM = 64
NCORES = 8
BPC = B // NCORES  # 512 batch rows per core
LMAX = 3
NMULT = 4  # multiplicity of each l in '4x0e+4x1o+4x2e+4x3o'
LS = [l for l in range(LMAX + 1) for _ in range(NMULT)]

# block-diagonal packing of the 16 (l1,l2) pair matrices into 2 stationaries
PAIRS_A = [(3, 3), (3, 2), (2, 3), (1, 1)]
PAIRS_B = [(2, 2), (1, 3), (3, 1), (1, 2), (2, 1), (0, 3), (3, 0),
           (0, 2), (2, 0), (0, 1), (1, 0), (0, 0)]

_decomp_cache = None
_nc_cache = None


def _col_start(l, u):
    return sum((2 * ll + 1) * NMULT for ll in range(l)) + u * (2 * l + 1)


def _build_decomp():
    """Index bookkeeping only (no numerics): which cb entries form the two
    stationary matrices, which in1/in2 columns feed each partition row,
    and which output row h each psum row maps to."""
    global _decomp_cache
    if _decomp_cache is not None:
        return _decomp_cache

    # replicate build_cb_matrix's row layout
    layout = {}
    idx1 = 0
    for l1 in LS:
        idx2 = 0
        for l2 in LS:
            for l3 in range(abs(l1 - l2), l1 + l2 + 1):
                layout.setdefault(l3, []).append((l1, l2, idx1 * DIM + idx2))
            idx2 += 2 * l2 + 1
        idx1 += 2 * l1 + 1
    entry_row = {}
    row = 0
    for l3 in sorted(layout):
        for (l1, l2, co) in sorted(layout[l3], key=lambda x: x[0] * LMAX + x[1]):
            entry_row[(l3, co)] = row
            row += 2 * l3 + 1
    assert row == B

    groups = []
    for pairs in (PAIRS_A, PAIRS_B):
        assert sum((2 * a + 1) * (2 * b + 1) for a, b in pairs) == 128
        c1 = np.zeros((NMULT, 128), dtype=np.int64)
        c2 = np.zeros((NMULT, 128), dtype=np.int64)
        h_of = np.zeros((NMULT, NMULT, 128), dtype=np.int64)
        w_k, w_m, w_h, w_c = [], [], [], []  # W[k,m] = cb[h, c]
        off = 0
        for (l1, l2) in pairs:
            n1, n2 = 2 * l1 + 1, 2 * l2 + 1
            kp = n1 * n2
            kk = np.arange(kp)
            m1, m2 = kk // n2, kk % n2
            for u in range(NMULT):
                c1[u, off:off + kp] = _col_start(l1, u) + m1
            for v in range(NMULT):
                c2[v, off:off + kp] = _col_start(l2, v) + m2
            mm = 0
            for l3 in range(abs(l1 - l2), l1 + l2 + 1):
                n3 = 2 * l3 + 1
                h0 = entry_row[(l3, _col_start(l1, 0) * DIM + _col_start(l2, 0))]
                km, m3m = np.meshgrid(kk, np.arange(n3), indexing="ij")
                w_k.append((off + km).ravel())
                w_m.append((off + mm + m3m).ravel())
                w_h.append((h0 + m3m).ravel())
                w_c.append(((_col_start(l1, 0) + m1[km.ravel()]) * DIM
                            + (_col_start(l2, 0) + m2[km.ravel()])))
                for u in range(NMULT):
                    for v in range(NMULT):
                        h = entry_row[(l3, _col_start(l1, u) * DIM + _col_start(l2, v))]
                        h_of[u, v, off + mm:off + mm + n3] = np.arange(h, h + n3)
                mm += n3
            off += kp
        groups.append({
            "c1": c1, "c2": c2, "h_of": h_of,
            "w_k": np.concatenate(w_k), "w_m": np.concatenate(w_m),
            "w_h": np.concatenate(w_h), "w_c": np.concatenate(w_c),
        })

    # global output row -> h map: tile t = S*16 + u*4 + v holds rows
    # t*128 + mm  ->  h_of[S][u, v, mm]
    hglob = np.zeros(32 * 128, dtype=np.int64)
    for s, g in enumerate(groups):
        for u in range(NMULT):
            for v in range(NMULT):
                t = s * 16 + u * 4 + v
                hglob[t * 128:(t + 1) * 128] = g["h_of"][u, v]
    _decomp_cache = (groups, hglob)
    return _decomp_cache


def _split_waits(bir_bytes):
    """This container's walrus build rejects >1 sync-wait per instruction
    ("Too many sync wait commands"). Hoist extra waits onto standalone
    EventSemaphore instructions on the same engine (same lowering raw
    bass wait_ge uses)."""
    bir = json.loads(bir_bytes)
    n = 0
    for fn in bir["functions"]:
        for blk in fn["blocks"]:
            out = []
            for inst in blk["instructions"]:
                si = inst.get("sync_info")
                waits = (si or {}).get("on_wait") or []
                if len(waits) > 1:
                    for w in waits[:-1]:
                        n += 1
                        out.append({
                            "debug": inst.get("debug", 0),
                            "engine": inst["engine"],
                            "ins": [], "outs": [],
                            "name": f"I-wsplit-{n}",
                            "opcode": "EventSemaphore",
                            "sync_info": {"on_update": [], "on_wait": [w]},
                        })
                    si["on_wait"] = [waits[-1]]
                out.append(inst)
            blk["instructions"] = out
    return json.dumps(bir).encode()


def _build_nc():
    """Bass program, identical on all 8 cores (SPMD; per-core data differs).

    Inputs per core (1.4 MB total instead of shipping pre-gathered 4.2 MB):
      w [128, 256]    W_A | W_B stationaries
      x [128, 512]    in1T (rows 0..63) stacked on in2T (rows 64..127)
      r [128, 2048]   16 replication 0/1 stationaries R_q [128,128];
                      q = s*8 + {0..3: g1 u, 4..7: g2 v}
    The gathered tiles g = R_q.T @ x are built on the PE (its SBUF ports
    are separate from the DMA fabric), copied PSUM->SBUF, elementwise
    multiplied (DVE, fused 4-wide via 0-stride broadcast), matmul'd with
    W, copied PSUM->SBUF (DVE/ACT split) and DMA'd out.
    """
    global _nc_cache
    if _nc_cache is not None:
        return _nc_cache
    import os
    import concourse.bass as bass
    import concourse.mybir as mybir
    from concourse.bass import ts
    from concourse.tile import TileContext

    f32 = mybir.dt.float32
    nc = bass.Bass()
    w = nc.dram_tensor("w", [128, 256], f32, kind="ExternalInput")
    x = nc.dram_tensor("x", [128, BPC], f32, kind="ExternalInput")
    r = nc.dram_tensor("r", [128, 16 * 128], f32, kind="ExternalInput")
    o = nc.dram_tensor("o", [16, 128, 2 * BPC], f32, kind="ExternalOutput")

    n_warm = int(os.environ.get("KERNEL_WARMUP", "6"))

    def bcast4(ap):
        return ap.rearrange("p (a f) -> p a f", a=1).broadcast_to((128, 4, BPC))

    with TileContext(nc) as tc:
        with (
            tc.tile_pool(name="const", bufs=1) as cpool,
            tc.tile_pool(name="rhspool", bufs=3) as rhspool,
            tc.tile_pool(name="gps", bufs=2, space="PSUM") as gps,
            tc.tile_pool(name="mps", bufs=3, space="PSUM") as mps,
            tc.tile_pool(name="opool", bufs=4) as opool,
        ):
            xt = cpool.tile([128, BPC], f32, tag="x")
            nc.sync.dma_start(out=xt, in_=x[:, :])
            wt = cpool.tile([128, 256], f32, tag="w")
            nc.sync.dma_start(out=wt, in_=w[:, :])
            rt = cpool.tile([128, 16 * 128], f32, tag="r")
            nc.sync.dma_start(out=rt, in_=r[:, :])

            # load the ACT function table now, not lazily mid-kernel
            scratch = cpool.tile([128, 16], f32, tag="scratch")
            nc.scalar.copy(out=scratch, in_=wt[:, 0:16])

            # PE/HAM warmup
            if n_warm:
                wp = mps.tile([128, 2 * BPC], f32, tag="ps")
                for i in range(n_warm):
                    nc.tensor.matmul(wp[:, 0:256], wt[:, 0:128], wt,
                                     start=True, stop=True)

            # replication: g tiles = R_q.T @ x
            g1t = [None] * 8   # (s,u) -> [128, 512] tile
            g2t = [None] * 2   # s -> [128, 2048] tile (v slices)
            for s in range(2):
                g2t[s] = cpool.tile([128, 4 * BPC], f32, tag=f"g2_{s}", name=f"g2_{s}")
            for q in range(16):
                s, which, i = q // 8, (q % 8) // 4, q % 4
                ps = gps.tile([128, BPC], f32, tag="gps")
                nc.tensor.matmul(ps, rt[:, ts(q, 128)], xt,
                                 start=True, stop=True)
                if which == 0:
                    t = cpool.tile([128, BPC], f32, tag=f"g1_{q}", name=f"g1_{q}")
                    g1t[s * 4 + i] = t
                    dst = t[:, :]
                else:
                    dst = g2t[s][:, ts(i, BPC)]
                if q < 8:
                    nc.vector.tensor_copy(out=dst, in_=ps)
                else:
                    nc.scalar.copy(out=dst, in_=ps)

            for s in range(2):
                for u in range(NMULT):
                    rhs = rhspool.tile([128, 4 * BPC], f32, tag="rhs")
                    nc.vector.tensor_mul(
                        out=rhs.rearrange("p (a f) -> p a f", a=4),
                        in0=bcast4(g1t[s * 4 + u][:, :]),
                        in1=g2t[s].rearrange("p (a f) -> p a f", a=4))
                    for v in range(NMULT):
                        t = s * 16 + u * 4 + v
                        c, j = t // 2, t % 2
                        if j == 0:
                            ps = mps.tile([128, 2 * BPC], f32, tag="ps")
                        nc.tensor.matmul(ps[:, ts(j, BPC)], wt[:, ts(s, 128)],
                                         rhs[:, ts(v, BPC)],
                                         start=True, stop=True)
                        if j == 1:
                            ot = opool.tile([128, 2 * BPC], f32, tag="ot")
                            if c % 8 == 7:
                                nc.vector.tensor_copy(out=ot, in_=ps)
                            else:
                                nc.scalar.copy(out=ot, in_=ps)
                            nc.sync.dma_start(out=o[c, :, :], in_=ot)

    orig = nc.to_json_bytes
    nc.to_json_bytes = lambda: _split_waits(orig())
    _nc_cache = nc
    return nc


def kernel(in1, in2, cb, _want_stats=False):
    from concourse.bass_utils import run_bass_kernel_spmd

    in1 = np.ascontiguousarray(np.asarray(in1, dtype=np.float32))
    in2 = np.ascontiguousarray(np.asarray(in2, dtype=np.float32))
    cb = np.asarray(cb, dtype=np.float32)
    groups, hglob = _build_decomp()

    # stationaries extracted straight from cb (no wigner math needed)
    wmat = np.zeros((2, 128, 128), dtype=np.float32)
    for s, g in enumerate(groups):
        wmat[s][g["w_k"], g["w_m"]] = cb[g["w_h"], g["w_c"]]

    wpack = np.ascontiguousarray(
        np.concatenate([wmat[0], wmat[1]], axis=1))  # [128, 256]

    # replication stationaries: q = s*8 + {0..3: g1 u, 4..7: g2 v}
    rmat = np.zeros((128, 16, 128), dtype=np.float32)
    cols = np.arange(128)
    for s, g in enumerate(groups):
        for i in range(NMULT):
            rmat[g["c1"][i], s * 8 + i, cols] = 1.0
            rmat[64 + g["c2"][i], s * 8 + 4 + i, cols] = 1.0
    rmat = np.ascontiguousarray(rmat.reshape(128, 16 * 128))

    in_maps = []
    for c in range(NCORES):
        sl = slice(c * BPC, (c + 1) * BPC)
        xs = np.ascontiguousarray(
            np.concatenate([in1[sl].T, in2[sl].T], axis=0))  # [128, 512]
        in_maps.append({"w": wpack, "x": xs, "r": rmat})

    nc = _build_nc()
    import os
    trace = bool(int(os.environ.get("KERNEL_TRACE", "0")))
    res = run_bass_kernel_spmd(nc, in_maps, core_ids=list(range(NCORES)),
                               trace=trace)

    # [4096 permuted rows, 4096 batch]; o is [16 chunks, 128, 2 tiles * 512]
    full = np.concatenate(
        [r["o"].reshape(16, 128, 2, BPC).transpose(0, 2, 1, 3).reshape(32 * 128, BPC)
         for r in res.results], axis=1)
    out = np.empty((B, B), dtype=np.float32)
    out[:, hglob] = full.T
    if _want_stats:
        return out, res
    return out


if __name__ == "__main__":
    rng = np.random.default_rng(0)
    a = rng.standard_normal((B, DIM)).astype(np.float32)
    b = rng.standard_normal((B, DIM)).astype(np.float32)
    cb = np.load("/tmp/cb.npy")
    out = kernel(a, b, cb)
    outer = np.einsum("bi,bj->bij", a, b).reshape(B, -1)
    exp = outer @ cb.T
    print("rel err:", np.linalg.norm(out - exp) / np.linalg.norm(exp))


# revision 17
# speedup vs baseline: 1.1219x; 1.1219x over previous
"""COOTensorProduct kernel for 8 Trainium2 NeuronCores.

Math: out[b, h] = sum_{i,j} cb[h, i*64+j] * in1[b, i] * in2[b, j]
with in1/in2 [4096, 64], cb [4096, 4096] (a Clebsch-Gordan / Wigner-3j
coupling matrix for irreps '4x0e+4x1o+4x2e+4x3o' x same -> all l3).

cb is 0.1% dense but perfectly block-structured: for each (l1, l2) pair
of irrep types the coupling is a square (2l1+1)(2l2+1) x (2l1+1)(2l2+1)
matrix (stacked l3 blocks), identical across the 4x4 multiplicity copies
(u, v). The 16 pair matrices have sizes {1,3,3,5,5,7,7,9,15,15,21,21,
25,35,35,49} which pack block-diagonally into exactly two 128x128
stationary matrices (49+35+35+9 = 128 and the rest = 128).

Per core (512 batch rows):
  rhs[S][u,v]  = in1T_gathered[S,u] * in2T_gathered[S,v]   (elementwise,
                 [128 partitions = (pair,m1,m2) rows, 512 free = batch])
  psum[S][u,v] = W_S.T @ rhs        (one 128x128x512 matmul)
so the whole problem is 32 elementwise mults + 32 matmuls per core.

Host does the (static, index-only) gathers/permutes; device does all
FLOPs. Output comes back as [4096 permuted rows, 512 batch] per core and
is un-permuted/transposed on host.
"""

import json
import numpy as np

# ---------------------------------------------------------------- problem
B = 4096
DIM = 64
NCORES = 8
BPC = B // NCORES  # 512 batch rows per core
LMAX = 3
NMULT = 4  # multiplicity of each l in '4x0e+4x1o+4x2e+4x3o'
LS = [l for l in range(LMAX + 1) for _ in range(NMULT)]

# block-diagonal packing of the 16 (l1,l2) pair matrices into 2 stationaries
PAIRS_A = [(3, 3), (3, 2), (2, 3), (1, 1)]
PAIRS_B = [(2, 2), (1, 3), (3, 1), (1, 2), (2, 1), (0, 3), (3, 0),
           (0, 2), (2, 0), (0, 1), (1, 0), (0, 0)]

_decomp_cache = None
_nc_cache = None


def _col_start(l, u):
    return sum((2 * ll + 1) * NMULT for ll in range(l)) + u * (2 * l + 1)


def _build_decomp():
    """Index bookkeeping only (no numerics): which cb entries form the two
    stationary matrices, which in1/in2 columns feed each partition row,
    and which output row h each psum row maps to."""
    global _decomp_cache
    if _decomp_cache is not None:
        return _decomp_cache

    # replicate build_cb_matrix's row layout
    layout = {}
    idx1 = 0
    for l1 in LS:
        idx2 = 0
        for l2 in LS:
            for l3 in range(abs(l1 - l2), l1 + l2 + 1):
                layout.setdefault(l3, []).append((l1, l2, idx1 * DIM + idx2))
            idx2 += 2 * l2 + 1
        idx1 += 2 * l1 + 1
    entry_row = {}
    row = 0
    for l3 in sorted(layout):
        for (l1, l2, co) in sorted(layout[l3], key=lambda x: x[0] * LMAX + x[1]):
            entry_row[(l3, co)] = row
            row += 2 * l3 + 1
    assert row == B

    groups = []
    for pairs in (PAIRS_A, PAIRS_B):
        assert sum((2 * a + 1) * (2 * b + 1) for a, b in pairs) == 128
        c1 = np.zeros((NMULT, 128), dtype=np.int64)
        c2 = np.zeros((NMULT, 128), dtype=np.int64)
        h_of = np.zeros((NMULT, NMULT, 128), dtype=np.int64)
        w_k, w_m, w_h, w_c = [], [], [], []  # W[k,m] = cb[h, c]
        off = 0
        for (l1, l2) in pairs:
            n1, n2 = 2 * l1 + 1, 2 * l2 + 1
            kp = n1 * n2
            kk = np.arange(kp)
            m1, m2 = kk // n2, kk % n2
            for u in range(NMULT):
                c1[u, off:off + kp] = _col_start(l1, u) + m1
            for v in range(NMULT):
                c2[v, off:off + kp] = _col_start(l2, v) + m2
            mm = 0
            for l3 in range(abs(l1 - l2), l1 + l2 + 1):
                n3 = 2 * l3 + 1
                h0 = entry_row[(l3, _col_start(l1, 0) * DIM + _col_start(l2, 0))]
                km, m3m = np.meshgrid(kk, np.arange(n3), indexing="ij")
                w_k.append((off + km).ravel())
                w_m.append((off + mm + m3m).ravel())
                w_h.append((h0 + m3m).ravel())
                w_c.append(((_col_start(l1, 0) + m1[km.ravel()]) * DIM
                            + (_col_start(l2, 0) + m2[km.ravel()])))
                for u in range(NMULT):
                    for v in range(NMULT):
                        h = entry_row[(l3, _col_start(l1, u) * DIM + _col_start(l2, v))]
                        h_of[u, v, off + mm:off + mm + n3] = np.arange(h, h + n3)
                mm += n3
            off += kp
        groups.append({
            "c1": c1, "c2": c2, "h_of": h_of,
            "w_k": np.concatenate(w_k), "w_m": np.concatenate(w_m),
            "w_h": np.concatenate(w_h), "w_c": np.concatenate(w_c),
        })

    # global output row -> h map: tile t = S*16 + u*4 + v holds rows
    # t*128 + mm  ->  h_of[S][u, v, mm]
    hglob = np.zeros(32 * 128, dtype=np.int64)
    for s, g in enumerate(groups):
        for u in range(NMULT):
            for v in range(NMULT):
                t = s * 16 + u * 4 + v
                hglob[t * 128:(t + 1) * 128] = g["h_of"][u, v]
    _decomp_cache = (groups, hglob)
    return _decomp_cache


def _split_waits(bir_bytes):
    """This container's walrus build rejects >1 sync-wait per instruction
    ("Too many sync wait commands"). Hoist extra waits onto standalone
    EventSemaphore instructions on the same engine (same lowering raw
    bass wait_ge uses)."""
    bir = json.loads(bir_bytes)
    n = 0
    for fn in bir["functions"]:
        for blk in fn["blocks"]:
            out = []
            for inst in blk["instructions"]:
                si = inst.get("sync_info")
                waits = (si or {}).get("on_wait") or []
                if len(waits) > 1:
                    for w in waits[:-1]:
                        n += 1
                        out.append({
                            "debug": inst.get("debug", 0),
                            "engine": inst["engine"],
                            "ins": [], "outs": [],
                            "name": f"I-wsplit-{n}",
                            "opcode": "EventSemaphore",
                            "sync_info": {"on_update": [], "on_wait": [w]},
                        })
                    si["on_wait"] = [waits[-1]]
                out.append(inst)
            blk["instructions"] = out
    return json.dumps(bir).encode()


def _build_nc():
    """Bass program, identical on all 8 cores (SPMD; per-core data differs).

    Inputs per core (1.4 MB instead of 4.2 MB pre-gathered):
      w [128, 256]    W_A | W_B stationaries
      x [128, 512]    in1T (rows 0..63) stacked on in2T (rows 64..127)
      r [128, 2048]   16 replication 0/1 stationaries R_q [128,128];
                      q = s*8 + {0..3: g1 u, 4..7: g2 v}
    Pipeline per phase s: 4 replication matmuls build g1 tiles in PSUM
    (consumed there directly by the DVE mults), 4 more build g2 tiles
    (copied to SBUF); fused 4-wide mults make rhs; W_s matmuls produce
    the 16 output tiles; DVE/ACT copy PSUM->SBUF; DMA out.
    Matmuls run as float32r (1 cycle/row vs fp32's 4) unless
    KERNEL_F32R=0.
    """
    global _nc_cache
    if _nc_cache is not None:
        return _nc_cache
    import os
    import concourse.bass as bass
    import concourse.mybir as mybir
    from concourse.bass import ts
    from concourse.tile import TileContext

    f32 = mybir.dt.float32
    f32r = mybir.dt.float32r
    use_f32r = bool(int(os.environ.get("KERNEL_F32R", "1")))
    n_warm = int(os.environ.get("KERNEL_WARMUP", "0"))

    dmm = f32r if use_f32r else f32

    nc = bass.Bass()
    w = nc.dram_tensor("w", [128, 256], dmm, kind="ExternalInput")
    x = nc.dram_tensor("x", [128, BPC], dmm, kind="ExternalInput")
    r = nc.dram_tensor("r", [128, 16 * 128], dmm, kind="ExternalInput")
    o = nc.dram_tensor("o", [32, 128, BPC], f32, kind="ExternalOutput")

    def bcast4(ap):
        return ap.rearrange("p (a f) -> p a f", a=1).broadcast_to((128, 4, BPC))

    with TileContext(nc) as tc:
        with (
            tc.tile_pool(name="const", bufs=1) as cpool,
            tc.tile_pool(name="rhspool", bufs=3) as rhspool,
            tc.tile_pool(name="gps1", bufs=4, space="PSUM") as gps1,
            tc.tile_pool(name="gps2", bufs=1, space="PSUM") as gps2,
            tc.tile_pool(name="mps", bufs=3, space="PSUM") as mps,
            tc.tile_pool(name="opool", bufs=6) as opool,
        ):
            xt = cpool.tile([128, BPC], dmm, tag="x")
            nc.sync.dma_start(out=xt, in_=x[:, :])
            wt = cpool.tile([128, 256], dmm, tag="w")
            nc.sync.dma_start(out=wt, in_=w[:, :])
            rt = cpool.tile([128, 16 * 128], dmm, tag="r")
            nc.sync.dma_start(out=rt, in_=r[:, :])

            # load the ACT function table now, not lazily mid-kernel
            scratch = cpool.tile([128, 16], f32, tag="scratch")
            nc.scalar.copy(out=scratch, in_=wt[:, 0:16])

            if n_warm:
                wp = mps.tile([128, BPC], f32, tag="ps")
                for i in range(n_warm):
                    nc.tensor.matmul(wp[:, 0:256], wt[:, 0:128], wt,
                                     start=True, stop=True)

            g2t = [None] * 2
            for s in range(2):
                g2t[s] = cpool.tile([128, 4 * BPC], f32, tag=f"g2_{s}",
                                    name=f"g2_{s}")

            for s in range(2):
                # replication: g1 stays in PSUM, g2 lands in SBUF via ACT
                g1ps = []
                for i in range(NMULT):
                    ps = gps1.tile([128, BPC], f32, tag="g1ps")
                    nc.tensor.matmul(ps, rt[:, ts(s * 8 + i, 128)],
                                     xt, start=True, stop=True)
                    g1ps.append(ps)
                for i in range(NMULT):
                    ps = gps2.tile([128, BPC], f32, tag="g2ps")
                    nc.tensor.matmul(ps, rt[:, ts(s * 8 + 4 + i, 128)],
                                     xt, start=True, stop=True)
                    nc.scalar.copy(out=g2t[s][:, ts(i, BPC)], in_=ps)

                for u in range(NMULT):
                    rhs = rhspool.tile([128, 4 * BPC], dmm, tag="rhs")
                    nc.vector.tensor_mul(
                        out=rhs.rearrange("p (a f) -> p a f", a=4),
                        in0=bcast4(g1ps[u][:, :]),
                        in1=g2t[s].rearrange("p (a f) -> p a f", a=4))
                    for v in range(NMULT):
                        t = s * 16 + u * 4 + v
                        ps = mps.tile([128, BPC], f32, tag="ps")
                        nc.tensor.matmul(ps, wt[:, ts(s, 128)],
                                         rhs[:, ts(v, BPC)],
                                         start=True, stop=True)
                        ot = opool.tile([128, BPC], f32, tag="ot")
                        if t % 4 == 3:
                            nc.vector.tensor_copy(out=ot, in_=ps)
                        else:
                            nc.scalar.copy(out=ot, in_=ps)
                        nc.sync.dma_start(out=o[t, :, :], in_=ot)

    orig = nc.to_json_bytes
    nc.to_json_bytes = lambda: _split_waits(orig())
    _nc_cache = nc
    return nc


def kernel(in1, in2, cb, _want_stats=False):
    from concourse.bass_utils import run_bass_kernel_spmd

    in1 = np.ascontiguousarray(np.asarray(in1, dtype=np.float32))
    in2 = np.ascontiguousarray(np.asarray(in2, dtype=np.float32))
    cb = np.asarray(cb, dtype=np.float32)
    groups, hglob = _build_decomp()

    # stationaries extracted straight from cb (no wigner math needed)
    wmat = np.zeros((2, 128, 128), dtype=np.float32)
    for s, g in enumerate(groups):
        wmat[s][g["w_k"], g["w_m"]] = cb[g["w_h"], g["w_c"]]

    wpack = np.ascontiguousarray(
        np.concatenate([wmat[0], wmat[1]], axis=1))  # [128, 256]

    # replication stationaries: q = s*8 + {0..3: g1 u, 4..7: g2 v}
    rmat = np.zeros((128, 16, 128), dtype=np.float32)
    cols = np.arange(128)
    for s, g in enumerate(groups):
        for i in range(NMULT):
            rmat[g["c1"][i], s * 8 + i, cols] = 1.0
            rmat[64 + g["c2"][i], s * 8 + 4 + i, cols] = 1.0
    rmat = np.ascontiguousarray(rmat.reshape(128, 16 * 128))

    in_maps = []
    for c in range(NCORES):
        sl = slice(c * BPC, (c + 1) * BPC)
        xs = np.ascontiguousarray(
            np.concatenate([in1[sl].T, in2[sl].T], axis=0))  # [128, 512]
        in_maps.append({"w": wpack, "x": xs, "r": rmat})

    nc = _build_nc()
    import os
    trace = bool(int(os.environ.get("KERNEL_TRACE", "0")))
    res = run_bass_kernel_spmd(nc, in_maps, core_ids=list(range(NCORES)),
                               trace=trace)

    # [4096 permuted rows, 4096 batch]; o is [32 tiles, 128, 512]
    full = np.concatenate(
        [r["o"].reshape(32 * 128, BPC) for r in res.results], axis=1)
    out = np.empty((B, B), dtype=np.float32)
    out[:, hglob] = full.T
    if _want_stats:
        return out, res
    return out


if __name__ == "__main__":
    rng = np.random.default_rng(0)
    a = rng.standard_normal((B, DIM)).astype(np.float32)
    b = rng.standard_normal((B, DIM)).astype(np.float32)
    cb = np.load("/tmp/cb.npy")
    out = kernel(a, b, cb)
    outer = np.einsum("bi,bj->bij", a, b).reshape(B, -1)
    exp = outer @ cb.T
    print("rel err:", np.linalg.norm(out - exp) / np.linalg.norm(exp))


# revision 19
# speedup vs baseline: 1.2129x; 1.0811x over previous
"""COOTensorProduct kernel for 8 Trainium2 NeuronCores.

Math: out[b, h] = sum_{i,j} cb[h, i*64+j] * in1[b, i] * in2[b, j]
with in1/in2 [4096, 64], cb [4096, 4096] (a Clebsch-Gordan / Wigner-3j
coupling matrix for irreps '4x0e+4x1o+4x2e+4x3o' x same -> all l3).

cb is 0.1% dense but perfectly block-structured: for each (l1, l2) pair
of irrep types the coupling is a square (2l1+1)(2l2+1) x (2l1+1)(2l2+1)
matrix (stacked l3 blocks), identical across the 4x4 multiplicity copies
(u, v). The 16 pair matrices have sizes {1,3,3,5,5,7,7,9,15,15,21,21,
25,35,35,49} which pack block-diagonally into exactly two 128x128
stationary matrices (49+35+35+9 = 128 and the rest = 128).

Per core (512 batch rows):
  rhs[S][u,v]  = in1T_gathered[S,u] * in2T_gathered[S,v]   (elementwise,
                 [128 partitions = (pair,m1,m2) rows, 512 free = batch])
  psum[S][u,v] = W_S.T @ rhs        (one 128x128x512 matmul)
so the whole problem is 32 elementwise mults + 32 matmuls per core.

Host does the (static, index-only) gathers/permutes; device does all
FLOPs. Output comes back as [4096 permuted rows, 512 batch] per core and
is un-permuted/transposed on host.
"""

import json
import numpy as np

# ---------------------------------------------------------------- problem
B = 4096
DIM = 64
NCORES = 8
BPC = B // NCORES  # 512 batch rows per core
LMAX = 3
NMULT = 4  # multiplicity of each l in '4x0e+4x1o+4x2e+4x3o'
LS = [l for l in range(LMAX + 1) for _ in range(NMULT)]

# block-diagonal packing of the 16 (l1,l2) pair matrices into 2 stationaries
PAIRS_A = [(3, 3), (3, 2), (2, 3), (1, 1)]
PAIRS_B = [(2, 2), (1, 3), (3, 1), (1, 2), (2, 1), (0, 3), (3, 0),
           (0, 2), (2, 0), (0, 1), (1, 0), (0, 0)]

_decomp_cache = None
_nc_cache = None


def _col_start(l, u):
    return sum((2 * ll + 1) * NMULT for ll in range(l)) + u * (2 * l + 1)


def _build_decomp():
    """Index bookkeeping only (no numerics): which cb entries form the two
    stationary matrices, which in1/in2 columns feed each partition row,
    and which output row h each psum row maps to."""
    global _decomp_cache
    if _decomp_cache is not None:
        return _decomp_cache

    # replicate build_cb_matrix's row layout
    layout = {}
    idx1 = 0
    for l1 in LS:
        idx2 = 0
        for l2 in LS:
            for l3 in range(abs(l1 - l2), l1 + l2 + 1):
                layout.setdefault(l3, []).append((l1, l2, idx1 * DIM + idx2))
            idx2 += 2 * l2 + 1
        idx1 += 2 * l1 + 1
    entry_row = {}
    row = 0
    for l3 in sorted(layout):
        for (l1, l2, co) in sorted(layout[l3], key=lambda x: x[0] * LMAX + x[1]):
            entry_row[(l3, co)] = row
            row += 2 * l3 + 1
    assert row == B

    groups = []
    for pairs in (PAIRS_A, PAIRS_B):
        assert sum((2 * a + 1) * (2 * b + 1) for a, b in pairs) == 128
        c1 = np.zeros((NMULT, 128), dtype=np.int64)
        c2 = np.zeros((NMULT, 128), dtype=np.int64)
        h_of = np.zeros((NMULT, NMULT, 128), dtype=np.int64)
        w_k, w_m, w_h, w_c = [], [], [], []  # W[k,m] = cb[h, c]
        off = 0
        for (l1, l2) in pairs:
            n1, n2 = 2 * l1 + 1, 2 * l2 + 1
            kp = n1 * n2
            kk = np.arange(kp)
            m1, m2 = kk // n2, kk % n2
            for u in range(NMULT):
                c1[u, off:off + kp] = _col_start(l1, u) + m1
            for v in range(NMULT):
                c2[v, off:off + kp] = _col_start(l2, v) + m2
            mm = 0
            for l3 in range(abs(l1 - l2), l1 + l2 + 1):
                n3 = 2 * l3 + 1
                h0 = entry_row[(l3, _col_start(l1, 0) * DIM + _col_start(l2, 0))]
                km, m3m = np.meshgrid(kk, np.arange(n3), indexing="ij")
                w_k.append((off + km).ravel())
                w_m.append((off + mm + m3m).ravel())
                w_h.append((h0 + m3m).ravel())
                w_c.append(((_col_start(l1, 0) + m1[km.ravel()]) * DIM
                            + (_col_start(l2, 0) + m2[km.ravel()])))
                for u in range(NMULT):
                    for v in range(NMULT):
                        h = entry_row[(l3, _col_start(l1, u) * DIM + _col_start(l2, v))]
                        h_of[u, v, off + mm:off + mm + n3] = np.arange(h, h + n3)
                mm += n3
            off += kp
        groups.append({
            "c1": c1, "c2": c2, "h_of": h_of,
            "w_k": np.concatenate(w_k), "w_m": np.concatenate(w_m),
            "w_h": np.concatenate(w_h), "w_c": np.concatenate(w_c),
        })

    # global output row -> h map: tile t = S*16 + u*4 + v holds rows
    # t*128 + mm  ->  h_of[S][u, v, mm]
    hglob = np.zeros(32 * 128, dtype=np.int64)
    for s, g in enumerate(groups):
        for u in range(NMULT):
            for v in range(NMULT):
                t = s * 16 + u * 4 + v
                hglob[t * 128:(t + 1) * 128] = g["h_of"][u, v]
    _decomp_cache = (groups, hglob)
    return _decomp_cache


def _split_waits(bir_bytes):
    """This container's walrus build rejects >1 sync-wait per instruction
    ("Too many sync wait commands"). Hoist extra waits onto standalone
    EventSemaphore instructions on the same engine (same lowering raw
    bass wait_ge uses)."""
    bir = json.loads(bir_bytes)
    n = 0
    for fn in bir["functions"]:
        for blk in fn["blocks"]:
            out = []
            for inst in blk["instructions"]:
                si = inst.get("sync_info")
                waits = (si or {}).get("on_wait") or []
                if len(waits) > 1:
                    for w in waits[:-1]:
                        n += 1
                        out.append({
                            "debug": inst.get("debug", 0),
                            "engine": inst["engine"],
                            "ins": [], "outs": [],
                            "name": f"I-wsplit-{n}",
                            "opcode": "EventSemaphore",
                            "sync_info": {"on_update": [], "on_wait": [w]},
                        })
                    si["on_wait"] = [waits[-1]]
                out.append(inst)
            blk["instructions"] = out
    return json.dumps(bir).encode()


def _build_nc():
    """Bass program, identical on all 8 cores (SPMD; per-core data differs).

    Inputs per core (1.4 MB instead of 4.2 MB pre-gathered):
      w [128, 256]    W_A | W_B stationaries
      x [128, 512]    in1T (rows 0..63) stacked on in2T (rows 64..127)
      r [128, 2048]   16 replication 0/1 stationaries R_q [128,128];
                      q = s*8 + {0..3: g1 u, 4..7: g2 v}
    Pipeline per phase s: 4 replication matmuls build g1 tiles in PSUM
    (consumed there directly by the DVE mults), 4 more build g2 tiles
    (copied to SBUF); fused 4-wide mults make rhs; W_s matmuls produce
    the 16 output tiles; DVE/ACT copy PSUM->SBUF; DMA out.
    Matmuls run as float32r (1 cycle/row vs fp32's 4) unless
    KERNEL_F32R=0.
    """
    global _nc_cache
    if _nc_cache is not None:
        return _nc_cache
    import os
    import concourse.bass as bass
    import concourse.mybir as mybir
    from concourse.bass import ts
    from concourse.tile import TileContext

    f32 = mybir.dt.float32
    f32r = mybir.dt.float32r
    use_f32r = bool(int(os.environ.get("KERNEL_F32R", "1")))
    n_warm = int(os.environ.get("KERNEL_WARMUP", "0"))

    dmm = f32r if use_f32r else f32

    nc = bass.Bass()
    w = nc.dram_tensor("w", [128, 256], dmm, kind="ExternalInput")
    x = nc.dram_tensor("x", [128, BPC], dmm, kind="ExternalInput")
    r = nc.dram_tensor("r", [128, 16 * 128], dmm, kind="ExternalInput")
    o = nc.dram_tensor("o", [16, 128, 2 * BPC], f32, kind="ExternalOutput")

    def bcast2(ap):
        return ap.rearrange("p (a f) -> p a f", a=1).broadcast_to((128, 2, BPC))

    with TileContext(nc) as tc:
        with (
            tc.tile_pool(name="const", bufs=1) as cpool,
            tc.tile_pool(name="rhspool", bufs=3) as rhspool,
            tc.tile_pool(name="gps1", bufs=3, space="PSUM") as gps1,
            tc.tile_pool(name="gps2", bufs=1, space="PSUM") as gps2,
            tc.tile_pool(name="mps", bufs=2, space="PSUM") as mps,
            tc.tile_pool(name="opool", bufs=6) as opool,
        ):
            xt = cpool.tile([128, BPC], dmm, tag="x")
            nc.sync.dma_start(out=xt, in_=x[:, :])
            rt = cpool.tile([128, 16 * 128], dmm, tag="r")
            nc.sync.dma_start(out=rt[:, 0:8 * 128], in_=r[:, 0:8 * 128])
            wt = cpool.tile([128, 256], dmm, tag="w")
            nc.sync.dma_start(out=wt, in_=w[:, :])
            nc.sync.dma_start(out=rt[:, 8 * 128:], in_=r[:, 8 * 128:])

            # load the ACT function table now, not lazily mid-kernel
            scratch = cpool.tile([128, 16], f32, tag="scratch")
            nc.scalar.copy(out=scratch, in_=wt[:, 0:16])

            if n_warm:
                wp = mps.tile([128, BPC], f32, tag="ps")
                for i in range(n_warm):
                    nc.tensor.matmul(wp[:, 0:256], wt[:, 0:128], wt,
                                     start=True, stop=True)

            g2t = [None] * 2
            for s in range(2):
                g2t[s] = cpool.tile([128, 4 * BPC], f32, tag=f"g2_{s}",
                                    name=f"g2_{s}")

            for s in range(2):
                # replication: g1 stays in PSUM, g2 lands in SBUF via ACT
                for i in range(NMULT):
                    ps = gps2.tile([128, BPC], f32, tag="g2ps")
                    nc.tensor.matmul(ps, rt[:, ts(s * 8 + 4 + i, 128)],
                                     xt, start=True, stop=True)
                    nc.scalar.copy(out=g2t[s][:, ts(i, BPC)], in_=ps)
                g1ps = []
                for i in range(NMULT):
                    ps = gps1.tile([128, BPC], f32, tag="g1ps")
                    nc.tensor.matmul(ps, rt[:, ts(s * 8 + i, 128)],
                                     xt, start=True, stop=True)
                    g1ps.append(ps)

                for u in range(NMULT):
                    rhs = rhspool.tile([128, 4 * BPC], dmm, tag="rhs")
                    for h in range(2):
                        hs = slice(h * 2 * BPC, (h + 1) * 2 * BPC)
                        nc.vector.tensor_mul(
                            out=rhs[:, hs].rearrange("p (a f) -> p a f", a=2),
                            in0=bcast2(g1ps[u][:, :]),
                            in1=g2t[s][:, hs].rearrange(
                                "p (a f) -> p a f", a=2))
                    for v in range(NMULT):
                        t = s * 16 + u * 4 + v
                        if t % 2 == 0:
                            ps = mps.tile([128, 2 * BPC], f32, tag="ps")
                            ot = opool.tile([128, 2 * BPC], f32, tag="ot")
                        nc.tensor.matmul(ps[:, ts(t % 2, BPC)],
                                         wt[:, ts(s, 128)],
                                         rhs[:, ts(v, BPC)],
                                         start=True, stop=True)
                        if t % 4 == 3:
                            nc.vector.tensor_copy(out=ot[:, ts(1, BPC)],
                                                  in_=ps[:, ts(1, BPC)])
                        else:
                            nc.scalar.copy(out=ot[:, ts(t % 2, BPC)],
                                           in_=ps[:, ts(t % 2, BPC)])
                        if t % 2 == 1:
                            nc.sync.dma_start(out=o[t // 2, :, :], in_=ot)

    orig = nc.to_json_bytes
    nc.to_json_bytes = lambda: _split_waits(orig())
    _nc_cache = nc
    return nc


def kernel(in1, in2, cb, _want_stats=False):
    from concourse.bass_utils import run_bass_kernel_spmd

    in1 = np.ascontiguousarray(np.asarray(in1, dtype=np.float32))
    in2 = np.ascontiguousarray(np.asarray(in2, dtype=np.float32))
    cb = np.asarray(cb, dtype=np.float32)
    groups, hglob = _build_decomp()

    # stationaries extracted straight from cb (no wigner math needed)
    wmat = np.zeros((2, 128, 128), dtype=np.float32)
    for s, g in enumerate(groups):
        wmat[s][g["w_k"], g["w_m"]] = cb[g["w_h"], g["w_c"]]

    wpack = np.ascontiguousarray(
        np.concatenate([wmat[0], wmat[1]], axis=1))  # [128, 256]

    # replication stationaries: q = s*8 + {0..3: g1 u, 4..7: g2 v}
    rmat = np.zeros((128, 16, 128), dtype=np.float32)
    cols = np.arange(128)
    for s, g in enumerate(groups):
        for i in range(NMULT):
            rmat[g["c1"][i], s * 8 + i, cols] = 1.0
            rmat[64 + g["c2"][i], s * 8 + 4 + i, cols] = 1.0
    rmat = np.ascontiguousarray(rmat.reshape(128, 16 * 128))

    in_maps = []
    for c in range(NCORES):
        sl = slice(c * BPC, (c + 1) * BPC)
        xs = np.ascontiguousarray(
            np.concatenate([in1[sl].T, in2[sl].T], axis=0))  # [128, 512]
        in_maps.append({"w": wpack, "x": xs, "r": rmat})

    nc = _build_nc()
    import os
    trace = bool(int(os.environ.get("KERNEL_TRACE", "0")))
    res = run_bass_kernel_spmd(nc, in_maps, core_ids=list(range(NCORES)),
                               trace=trace)

    # [4096 permuted rows, 4096 batch]; o is [16 chunks, 128, 2 tiles * 512]
    full = np.concatenate(
        [r["o"].reshape(16, 128, 2, BPC).transpose(0, 2, 1, 3).reshape(32 * 128, BPC)
         for r in res.results], axis=1)
    out = np.empty((B, B), dtype=np.float32)
    out[:, hglob] = full.T
    if _want_stats:
        return out, res
    return out


if __name__ == "__main__":
    rng = np.random.default_rng(0)
    a = rng.standard_normal((B, DIM)).astype(np.float32)
    b = rng.standard_normal((B, DIM)).astype(np.float32)
    cb = np.load("/tmp/cb.npy")
    out = kernel(a, b, cb)
    outer = np.einsum("bi,bj->bij", a, b).reshape(B, -1)
    exp = outer @ cb.T
    print("rel err:", np.linalg.norm(out - exp) / np.linalg.norm(exp))
